# revision 1
# baseline (speedup 1.0000x reference)
"""DBRX MoE experts kernel for 8 Trainium2 NeuronCores.

Strategy (expert-parallel with host-side token dispatch):
  - Host computes the (cheap) router: softmax over 16 experts, top-4,
    renormalized gates.  Tokens are gathered per expert and packed into
    fixed-size "work items" of T tokens (zero-padded), each carrying its
    expert's weights.  Items are dealt evenly across the 8 cores.
  - Device (SPMD, one program on all 8 cores) runs the expert FFN for each
    item: h = wsT.T @ x (both halves), act = silu(h1)*h2, y = w2T.T @ act.
    All matmuls in float32r (full PE speed at N>=256, ~1e-4 rel err).
  - Host applies gates and scatter-adds item outputs into the final [T, D]
    output.  Only the FFN (97% of the FLOPs) runs on device; the dense
    16-expert reference computation is avoided entirely (4x FLOP saving).

Self-contained: hardcodes T=4096, D=1024, I=2048, E=16, top_k=4, 8 cores.
"""

import sys

if "/opt/trn_rl_repo" not in sys.path:
    sys.path.insert(0, "/opt/trn_rl_repo")

import numpy as np

import concourse.bacc as bacc
import concourse.mybir as mybir
import concourse.tile as tile
from concourse.bass_utils import run_bass_kernel_spmd

TOP_K = 4
N_CORES = 8
D = 1024
I = 2048
E = 16
DC = D // 128  # 8 contraction chunks for mm1 / output blocks for mm2
IC = I // 128  # 16 intermediate blocks
CB = 2 * I // 128  # 32 column blocks of ws

TRACE = False
LAST_EXEC_NS = None

_compiled = {}  # (NI, T) -> nc


def _build_program(NI, T):
    f32r = mybir.dt.float32r
    f32 = mybir.dt.float32
    nc = bacc.Bacc("TRN2", target_bir_lowering=False, debug=False, num_devices=N_CORES)

    # Layouts are pre-tiled on host so every DMA is partition-major contiguous.
    xT = nc.dram_tensor("xT", [NI, 128, DC, T], f32r, kind="ExternalInput")
    wsT = nc.dram_tensor("wsT", [NI, CB, 128, DC, 128], f32r, kind="ExternalInput")
    w2T = nc.dram_tensor("w2T", [NI, DC, 128, IC, 128], f32r, kind="ExternalInput")
    yT = nc.dram_tensor("yT", [NI, DC, 128, T], f32, kind="ExternalOutput")

    with tile.TileContext(nc) as tc:
        with (
            tc.tile_pool(name="xp", bufs=2) as xp,
            tc.tile_pool(name="wp", bufs=4) as wp,
            tc.tile_pool(name="w2p", bufs=3) as w2p,
            tc.tile_pool(name="actp", bufs=2) as actp,
            tc.tile_pool(name="sp", bufs=4) as sp,
            tc.tile_pool(name="pp", bufs=4, space="PSUM") as pp,
            tc.tile_pool(name="pp2", bufs=2, space="PSUM") as pp2,
        ):
            for it in range(NI):
                xt = xp.tile([128, DC, T], f32r, tag="x")
                nc.sync.dma_start(xt[:], xT.ap()[it])
                act = actp.tile([128, IC, T], f32r, tag="act")

                # mm1 + SwiGLU: h1 = w1 x, h2 = v1 x, act = silu(h1)*h2
                for ip in range(IC):
                    w1t = wp.tile([128, DC, 128], f32r, tag="ws")
                    nc.sync.dma_start(w1t[:], wsT.ap()[it, ip])
                    v1t = wp.tile([128, DC, 128], f32r, tag="ws")
                    nc.sync.dma_start(v1t[:], wsT.ap()[it, IC + ip])
                    ps1 = pp.tile([128, T], f32, tag="h")
                    ps2 = pp.tile([128, T], f32, tag="h")
                    for dc in range(DC):
                        nc.tensor.matmul(
                            ps1[:], w1t[:, dc], xt[:, dc],
                            start=(dc == 0), stop=(dc == DC - 1),
                        )
                    for dc in range(DC):
                        nc.tensor.matmul(
                            ps2[:], v1t[:, dc], xt[:, dc],
                            start=(dc == 0), stop=(dc == DC - 1),
                        )
                    st = sp.tile([128, T], f32, tag="silu")
                    nc.scalar.activation(
                        st[:], ps1[:], mybir.ActivationFunctionType.Silu
                    )
                    nc.vector.tensor_mul(act[:, ip], st[:], ps2[:])

                # mm2: y = w2 act
                for db in range(DC):
                    w2t = w2p.tile([128, IC, 128], f32r, tag="w2")
                    nc.sync.dma_start(w2t[:], w2T.ap()[it, db])
                    ps3 = pp2.tile([128, T], f32, tag="y")
                    for ic in range(IC):
                        nc.tensor.matmul(
                            ps3[:], w2t[:, ic], act[:, ic],
                            start=(ic == 0), stop=(ic == IC - 1),
                        )
                    ot = sp.tile([128, T], f32, tag="yout")
                    nc.any.tensor_copy(ot[:], ps3[:])
                    nc.sync.dma_start(yT.ap()[it, db], ot[:])
    nc.compile()
    return nc


def _routing(x, rw):
    logits = x @ rw.T
    m = logits.max(-1, keepdims=True)
    p = np.exp(logits - m)
    p /= p.sum(-1, keepdims=True)
    topk_idx = np.argpartition(-p, TOP_K - 1, axis=-1)[:, :TOP_K]
    topk_val = np.take_along_axis(p, topk_idx, -1)
    topk_val = topk_val / topk_val.sum(-1, keepdims=True)
    return topk_idx, topk_val


def _pick_T(counts):
    best = None
    for T in (512, 448, 384, 320, 256):
        items = int(sum((c + T - 1) // T for c in counts if c))
        NI = -(-items // 8)
        pe_ns = NI * 384 * T * 0.4167
        dma_bytes = NI * ((16 + 4) * 2**20 + 2 * 128 * DC * T * 4)
        dma_ns = dma_bytes / 0.36  # ~360 GB/s
        score = max(pe_ns, dma_ns)
        if best is None or score < best[0]:
            best = (score, T, NI)
    return best[1], best[2]


def _tile_ws(ws_e):
    # [cb, p, dc, col] = ws_e[cb*128+col, dc*128+p]
    return np.ascontiguousarray(
        ws_e.reshape(CB, 128, DC, 128).transpose(0, 3, 2, 1)
    )


def _tile_w2(w2_e):
    # [db, p, ic, col] = w2_e[db*128+col, ic*128+p]
    return np.ascontiguousarray(
        w2_e.reshape(DC, 128, IC, 128).transpose(0, 3, 2, 1)
    )


def kernel(hidden_states, router_w, ws, w2s):
    global LAST_EXEC_NS
    x = np.ascontiguousarray(np.asarray(hidden_states, dtype=np.float32))
    rw = np.asarray(router_w, dtype=np.float32)
    ws = np.asarray(ws, dtype=np.float32)
    w2s = np.asarray(w2s, dtype=np.float32)
    T_tok = x.shape[0]

    topk_idx, topk_val = _routing(x, rw)

    # per-expert token index lists
    expert_tok = []
    expert_gate = []
    for e in range(E):
        hit = topk_idx == e
        rows = np.nonzero(hit.any(-1))[0]
        gv = np.where(hit[rows], topk_val[rows], 0.0).sum(-1).astype(np.float32)
        expert_tok.append(rows)
        expert_gate.append(gv)

    counts = [len(t) for t in expert_tok]
    T, NI = _pick_T(counts)

    # build the global item list: (expert, token_idx_slice)
    items = []
    for e in range(E):
        toks = expert_tok[e]
        for s in range(0, len(toks), T):
            items.append((e, s, min(s + T, len(toks))))
    n_items = len(items)
    assert n_items <= NI * N_CORES

    ws_tiled = {}
    w2_tiled = {}
    for e in {e for e, _, _ in items}:
        ws_tiled[e] = _tile_ws(ws[e])
        w2_tiled[e] = _tile_w2(w2s[e])

    in_maps = []
    for c in range(N_CORES):
        xT_b = np.zeros((NI, 128, DC, T), dtype=np.float32)
        wsT_b = np.zeros((NI, CB, 128, DC, 128), dtype=np.float32)
        w2T_b = np.zeros((NI, DC, 128, IC, 128), dtype=np.float32)
        for slot in range(NI):
            gi = c * NI + slot
            if gi >= n_items:
                continue
            e, s, eend = items[gi]
            toks = expert_tok[e][s:eend]
            g = x[toks]  # [n, 1024]
            n = len(toks)
            xT_b[slot, :, :, :n] = g.reshape(n, DC, 128).transpose(2, 1, 0)
            wsT_b[slot] = ws_tiled[e]
            w2T_b[slot] = w2_tiled[e]
        in_maps.append({"xT": xT_b, "wsT": wsT_b, "w2T": w2T_b})

    key = (NI, T)
    if key not in _compiled:
        _compiled[key] = _build_program(NI, T)
    nc = _compiled[key]

    res = run_bass_kernel_spmd(
        nc, in_maps, core_ids=list(range(N_CORES)), trace=TRACE
    )
    LAST_EXEC_NS = res.exec_time_ns

    out = np.zeros((T_tok, D), dtype=np.float32)
    for gi, (e, s, eend) in enumerate(items):
        c, slot = divmod(gi, NI)
        toks = expert_tok[e][s:eend]
        n = len(toks)
        y_item = (
            res.results[c]["yT"][slot]
            .transpose(2, 0, 1)
            .reshape(T, D)[:n]
        )
        out[toks] += expert_gate[e][s:eend][:, None] * y_item
    return out


# revision 3
# speedup vs baseline: 1.2630x; 1.2630x over previous
"""DBRX MoE experts kernel for 8 Trainium2 NeuronCores.

Strategy (expert-parallel with host-side token dispatch):
  - Host computes the (cheap) router: softmax over 16 experts, top-4,
    renormalized gates.  Tokens are gathered per expert.
  - Each core gets NG=2 expert "groups" (16 experts / 8 cores); a group's
    tokens are packed into MG fixed-size tiles of T tokens (zero padded).
    The expert's weights are loaded once per group and reused across its
    MG token tiles.
  - Device (SPMD, one program on all 8 cores) runs the expert FFN:
    h = wsT.T @ x (both halves), act = silu(h1)*h2, y = w2T.T @ act.
    All matmuls in float32r (full PE speed at N>=256, ~2e-4 rel err).
  - Host applies gates and scatter-adds item outputs into the [T, D] output.
    Only the FFN (97% of the FLOPs) runs on device; the dense 16-expert
    reference computation is avoided entirely (4x FLOP saving via top-4).

Self-contained: hardcodes T=4096 tokens, D=1024, I=2048, E=16, top_k=4,
8 cores.
"""

import sys

if "/opt/trn_rl_repo" not in sys.path:
    sys.path.insert(0, "/opt/trn_rl_repo")

import numpy as np

import concourse.bacc as bacc
import concourse.mybir as mybir
import concourse.tile as tile
from concourse.bass_utils import run_bass_kernel_spmd

TOP_K = 4
N_CORES = 8
D = 1024
I = 2048
E = 16
DC = D // 128  # 8 contraction chunks for mm1 / output blocks for mm2
IC = I // 128  # 16 intermediate blocks
CB = 2 * I // 128  # 32 column blocks of ws

TRACE = False
LAST_EXEC_NS = None

_compiled = {}  # (NG, MG, T) -> nc


def _build_program(NG, MG, T):
    f32r = mybir.dt.float32r
    f32 = mybir.dt.float32
    nc = bacc.Bacc("TRN2", target_bir_lowering=False, debug=False, num_devices=N_CORES)

    NI = NG * MG
    # Layouts are pre-tiled on host so every DMA is partition-major contiguous.
    xT = nc.dram_tensor("xT", [NI, 128, DC, T], f32r, kind="ExternalInput")
    wsT = nc.dram_tensor("wsT", [NG, CB, 128, DC, 128], f32r, kind="ExternalInput")
    w2T = nc.dram_tensor("w2T", [NG, DC, 128, IC, 128], f32r, kind="ExternalInput")
    yT = nc.dram_tensor("yT", [NI, DC, 128, T], f32, kind="ExternalOutput")

    with tile.TileContext(nc) as tc:
        with (
            tc.tile_pool(name="xp", bufs=MG + 1) as xp,
            tc.tile_pool(name="wp", bufs=4) as wp,
            tc.tile_pool(name="w2p", bufs=3) as w2p,
            tc.tile_pool(name="actp", bufs=MG + 1) as actp,
            tc.tile_pool(name="sp", bufs=4) as sp,
            tc.tile_pool(name="pp", bufs=4, space="PSUM") as pp,
            tc.tile_pool(name="pp2", bufs=2, space="PSUM") as pp2,
        ):
            for g in range(NG):
                xts = []
                acts = []
                for j in range(MG):
                    xt = xp.tile([128, DC, T], f32r, tag="x")
                    nc.sync.dma_start(xt[:], xT.ap()[g * MG + j])
                    xts.append(xt)
                    act = actp.tile([128, IC, T], f32r, tag="act", name=f"act_{g}_{j}")
                    acts.append(act)

                # mm1 + SwiGLU: weights outer, token tiles inner (weight reuse)
                for ip in range(IC):
                    w1t = wp.tile([128, DC, 128], f32r, tag="ws")
                    nc.sync.dma_start(w1t[:], wsT.ap()[g, ip])
                    v1t = wp.tile([128, DC, 128], f32r, tag="ws")
                    nc.sync.dma_start(v1t[:], wsT.ap()[g, IC + ip])
                    for j in range(MG):
                        ps1 = pp.tile([128, T], f32, tag="h")
                        ps2 = pp.tile([128, T], f32, tag="h")
                        for dc in range(DC):
                            nc.tensor.matmul(
                                ps1[:], w1t[:, dc], xts[j][:, dc],
                                start=(dc == 0), stop=(dc == DC - 1),
                            )
                        for dc in range(DC):
                            nc.tensor.matmul(
                                ps2[:], v1t[:, dc], xts[j][:, dc],
                                start=(dc == 0), stop=(dc == DC - 1),
                            )
                        st = sp.tile([128, T], f32, tag="silu")
                        nc.scalar.activation(
                            st[:], ps1[:], mybir.ActivationFunctionType.Silu
                        )
                        nc.vector.tensor_mul(acts[j][:, ip], st[:], ps2[:])

                # mm2: w2 slabs outer, token tiles inner (weight reuse)
                for db in range(DC):
                    w2t = w2p.tile([128, IC, 128], f32r, tag="w2")
                    nc.sync.dma_start(w2t[:], w2T.ap()[g, db])
                    for j in range(MG):
                        ps3 = pp2.tile([128, T], f32, tag="y")
                        for ic in range(IC):
                            nc.tensor.matmul(
                                ps3[:], w2t[:, ic], acts[j][:, ic],
                                start=(ic == 0), stop=(ic == IC - 1),
                            )
                        ot = sp.tile([128, T], f32, tag="yout")
                        nc.any.tensor_copy(ot[:], ps3[:])
                        nc.sync.dma_start(yT.ap()[g * MG + j, db], ot[:])
    nc.compile()
    return nc


def _routing(x, rw):
    logits = x @ rw.T
    m = logits.max(-1, keepdims=True)
    p = np.exp(logits - m)
    p /= p.sum(-1, keepdims=True)
    topk_idx = np.argpartition(-p, TOP_K - 1, axis=-1)[:, :TOP_K]
    topk_val = np.take_along_axis(p, topk_idx, -1)
    topk_val = topk_val / topk_val.sum(-1, keepdims=True)
    return topk_idx, topk_val


def _pick_shape(counts):
    """Choose (NG, MG, T): NG expert groups per core, MG tiles of T tokens
    per group.  Every expert must fit in MG*T slots."""
    NG = -(-E // N_CORES)  # 2 for E=16
    cmax = max(counts)
    best = None
    for MG in range(1, 13):
        T = -(-cmax // MG)
        T = (T + 7) // 8 * 8  # align free dim to 32B
        if T > 512:
            continue
        if T < 256:
            break  # float32r matmul needs N>=256 for full PE speed
        pe = MG * T  # proportional to PE time
        if best is None or pe < best[0]:
            best = (pe, NG, MG, T)
    assert best is not None
    return best[1], best[2], best[3]


def _tile_ws(ws_e):
    # [cb, p, dc, col] = ws_e[cb*128+col, dc*128+p]
    return np.ascontiguousarray(
        ws_e.reshape(CB, 128, DC, 128).transpose(0, 3, 2, 1)
    )


def _tile_w2(w2_e):
    # [db, p, ic, col] = w2_e[db*128+col, ic*128+p]
    return np.ascontiguousarray(
        w2_e.reshape(DC, 128, IC, 128).transpose(0, 3, 2, 1)
    )


def kernel(hidden_states, router_w, ws, w2s):
    global LAST_EXEC_NS
    x = np.ascontiguousarray(np.asarray(hidden_states, dtype=np.float32))
    rw = np.asarray(router_w, dtype=np.float32)
    ws = np.asarray(ws, dtype=np.float32)
    w2s = np.asarray(w2s, dtype=np.float32)
    T_tok = x.shape[0]

    topk_idx, topk_val = _routing(x, rw)

    expert_tok = []
    expert_gate = []
    for e in range(E):
        hit = topk_idx == e
        rows = np.nonzero(hit.any(-1))[0]
        gv = np.where(hit[rows], topk_val[rows], 0.0).sum(-1).astype(np.float32)
        expert_tok.append(rows)
        expert_gate.append(gv)

    counts = [len(t) for t in expert_tok]
    NG, MG, T = _pick_shape(counts)
    NI = NG * MG

    key = (NG, MG, T)
    if key not in _compiled:
        _compiled[key] = _build_program(NG, MG, T)
    nc = _compiled[key]

    # expert -> (core, group): expert 2c+g goes to core c, group g
    in_maps = []
    for c in range(N_CORES):
        xT_b = np.zeros((NI, 128, DC, T), dtype=np.float32)
        wsT_b = np.empty((NG, CB, 128, DC, 128), dtype=np.float32)
        w2T_b = np.empty((NG, DC, 128, IC, 128), dtype=np.float32)
        for g in range(NG):
            e = NG * c + g
            wsT_b[g] = _tile_ws(ws[e])
            w2T_b[g] = _tile_w2(w2s[e])
            toks = expert_tok[e]
            for j in range(MG):
                seg = toks[j * T : (j + 1) * T]
                n = len(seg)
                if n == 0:
                    continue
                xT_b[g * MG + j, :, :, :n] = (
                    x[seg].reshape(n, DC, 128).transpose(2, 1, 0)
                )
        in_maps.append({"xT": xT_b, "wsT": wsT_b, "w2T": w2T_b})

    res = run_bass_kernel_spmd(
        nc, in_maps, core_ids=list(range(N_CORES)), trace=TRACE
    )
    LAST_EXEC_NS = res.exec_time_ns

    out = np.zeros((T_tok, D), dtype=np.float32)
    for e in range(E):
        c, g = divmod(e, NG)
        toks = expert_tok[e]
        gates = expert_gate[e]
        yT_c = res.results[c]["yT"]
        for j in range(MG):
            seg = toks[j * T : (j + 1) * T]
            n = len(seg)
            if n == 0:
                break
            y_item = yT_c[g * MG + j].transpose(2, 0, 1).reshape(T, D)[:n]
            out[seg] += gates[j * T : (j + 1) * T][:, None] * y_item
    return out


# revision 4
# speedup vs baseline: 1.2642x; 1.0009x over previous
"""DBRX MoE experts kernel for 8 Trainium2 NeuronCores.

Strategy (expert-parallel with host-side token dispatch):
  - Host computes the (cheap) router: softmax over 16 experts, top-4,
    renormalized gates.  Tokens are gathered per expert.
  - Each core gets NG=2 expert "groups" (16 experts / 8 cores); a group's
    tokens are packed into MG fixed-size tiles of T tokens (zero padded).
    The expert's weights are loaded once per group and reused across its
    MG token tiles.
  - Device (SPMD, one program on all 8 cores) runs the expert FFN:
    h = wsT.T @ x (both halves), act = silu(h1)*h2, y = w2T.T @ act.
    All matmuls in float32r (full PE speed at N>=256, ~2e-4 rel err).
  - Host applies gates and scatter-adds item outputs into the [T, D] output.
    Only the FFN (97% of the FLOPs) runs on device; the dense 16-expert
    reference computation is avoided entirely (4x FLOP saving via top-4).

Self-contained: hardcodes T=4096 tokens, D=1024, I=2048, E=16, top_k=4,
8 cores.
"""

import sys

if "/opt/trn_rl_repo" not in sys.path:
    sys.path.insert(0, "/opt/trn_rl_repo")

import numpy as np

import concourse.bacc as bacc
import concourse.mybir as mybir
import concourse.tile as tile
from concourse.bass_utils import run_bass_kernel_spmd

TOP_K = 4
N_CORES = 8
D = 1024
I = 2048
E = 16
DC = D // 128  # 8 contraction chunks for mm1 / output blocks for mm2
IC = I // 128  # 16 intermediate blocks
CB = 2 * I // 128  # 32 column blocks of ws

TRACE = False
LAST_EXEC_NS = None

_compiled = {}  # (NG, MG, T) -> nc


def _build_program(NG, MG, T):
    f32r = mybir.dt.float32r
    f32 = mybir.dt.float32
    nc = bacc.Bacc("TRN2", target_bir_lowering=False, debug=False, num_devices=N_CORES)

    NI = NG * MG
    # Layouts are pre-tiled on host so every DMA is partition-major contiguous.
    xT = nc.dram_tensor("xT", [NI, 128, DC, T], f32r, kind="ExternalInput")
    wsT = nc.dram_tensor("wsT", [NG, CB, 128, DC, 128], f32r, kind="ExternalInput")
    w2T = nc.dram_tensor("w2T", [NG, DC, 128, IC, 128], f32r, kind="ExternalInput")
    yT = nc.dram_tensor("yT", [NI, DC, 128, T], f32, kind="ExternalOutput")

    with tile.TileContext(nc) as tc:
        with (
            tc.tile_pool(name="xp", bufs=MG + 1) as xp,
            tc.tile_pool(name="wp", bufs=4) as wp,
            tc.tile_pool(name="w2p", bufs=3) as w2p,
            tc.tile_pool(name="actp", bufs=MG + 1) as actp,
            tc.tile_pool(name="sp", bufs=4) as sp,
            tc.tile_pool(name="pp", bufs=4, space="PSUM") as pp,
            tc.tile_pool(name="pp2", bufs=2, space="PSUM") as pp2,
        ):
            for g in range(NG):
                # first weight pair ahead of the (larger) x loads so the
                # critical-path DMAs for the first matmul overlap
                w1t0 = wp.tile([128, DC, 128], f32r, tag="ws", name=f"w1t0_{g}")
                nc.sync.dma_start(w1t0[:], wsT.ap()[g, 0])
                v1t0 = wp.tile([128, DC, 128], f32r, tag="ws", name=f"v1t0_{g}")
                nc.sync.dma_start(v1t0[:], wsT.ap()[g, IC])
                xts = []
                acts = []
                for j in range(MG):
                    xt = xp.tile([128, DC, T], f32r, tag="x", name=f"x_{g}_{j}")
                    # split per d-chunk: one big DMA bottlenecks on a single
                    # DMA queue (~90 GB/s); 8 chunk DMAs spread across queues
                    for dc in range(DC):
                        nc.sync.dma_start(xt[:, dc], xT.ap()[g * MG + j, :, dc])
                    xts.append(xt)
                    act = actp.tile([128, IC, T], f32r, tag="act", name=f"act_{g}_{j}")
                    acts.append(act)

                # mm1 + SwiGLU: weights outer, token tiles inner (weight reuse)
                for ip in range(IC):
                    if ip == 0:
                        w1t, v1t = w1t0, v1t0
                    else:
                        w1t = wp.tile([128, DC, 128], f32r, tag="ws")
                        nc.sync.dma_start(w1t[:], wsT.ap()[g, ip])
                        v1t = wp.tile([128, DC, 128], f32r, tag="ws")
                        nc.sync.dma_start(v1t[:], wsT.ap()[g, IC + ip])
                    for j in range(MG):
                        ps1 = pp.tile([128, T], f32, tag="h")
                        ps2 = pp.tile([128, T], f32, tag="h")
                        for dc in range(DC):
                            nc.tensor.matmul(
                                ps1[:], w1t[:, dc], xts[j][:, dc],
                                start=(dc == 0), stop=(dc == DC - 1),
                            )
                        for dc in range(DC):
                            nc.tensor.matmul(
                                ps2[:], v1t[:, dc], xts[j][:, dc],
                                start=(dc == 0), stop=(dc == DC - 1),
                            )
                        st = sp.tile([128, T], f32, tag="silu")
                        nc.scalar.activation(
                            st[:], ps1[:], mybir.ActivationFunctionType.Silu
                        )
                        nc.vector.tensor_mul(acts[j][:, ip], st[:], ps2[:])

                # mm2: w2 slabs outer, token tiles inner (weight reuse)
                for db in range(DC):
                    w2t = w2p.tile([128, IC, 128], f32r, tag="w2")
                    nc.sync.dma_start(w2t[:], w2T.ap()[g, db])
                    for j in range(MG):
                        ps3 = pp2.tile([128, T], f32, tag="y")
                        for ic in range(IC):
                            nc.tensor.matmul(
                                ps3[:], w2t[:, ic], acts[j][:, ic],
                                start=(ic == 0), stop=(ic == IC - 1),
                            )
                        ot = sp.tile([128, T], f32, tag="yout")
                        nc.any.tensor_copy(ot[:], ps3[:])
                        nc.sync.dma_start(yT.ap()[g * MG + j, db], ot[:])
    nc.compile()
    return nc


def _routing(x, rw):
    logits = x @ rw.T
    m = logits.max(-1, keepdims=True)
    p = np.exp(logits - m)
    p /= p.sum(-1, keepdims=True)
    topk_idx = np.argpartition(-p, TOP_K - 1, axis=-1)[:, :TOP_K]
    topk_val = np.take_along_axis(p, topk_idx, -1)
    topk_val = topk_val / topk_val.sum(-1, keepdims=True)
    return topk_idx, topk_val


def _pick_shape(counts):
    """Choose (NG, MG, T): NG expert groups per core, MG tiles of T tokens
    per group.  Every expert must fit in MG*T slots."""
    NG = -(-E // N_CORES)  # 2 for E=16
    cmax = max(counts)
    best = None
    for MG in range(1, 13):
        T = -(-cmax // MG)
        T = (T + 7) // 8 * 8  # align free dim to 32B
        if T > 512:
            continue
        if T < 256:
            break  # float32r matmul needs N>=256 for full PE speed
        pe = MG * T  # proportional to PE time
        if best is None or pe < best[0]:
            best = (pe, NG, MG, T)
    assert best is not None
    return best[1], best[2], best[3]


def _tile_ws(ws_e):
    # [cb, p, dc, col] = ws_e[cb*128+col, dc*128+p]
    return np.ascontiguousarray(
        ws_e.reshape(CB, 128, DC, 128).transpose(0, 3, 2, 1)
    )


def _tile_w2(w2_e):
    # [db, p, ic, col] = w2_e[db*128+col, ic*128+p]
    return np.ascontiguousarray(
        w2_e.reshape(DC, 128, IC, 128).transpose(0, 3, 2, 1)
    )


def kernel(hidden_states, router_w, ws, w2s):
    global LAST_EXEC_NS
    x = np.ascontiguousarray(np.asarray(hidden_states, dtype=np.float32))
    rw = np.asarray(router_w, dtype=np.float32)
    ws = np.asarray(ws, dtype=np.float32)
    w2s = np.asarray(w2s, dtype=np.float32)
    T_tok = x.shape[0]

    topk_idx, topk_val = _routing(x, rw)

    expert_tok = []
    expert_gate = []
    for e in range(E):
        hit = topk_idx == e
        rows = np.nonzero(hit.any(-1))[0]
        gv = np.where(hit[rows], topk_val[rows], 0.0).sum(-1).astype(np.float32)
        expert_tok.append(rows)
        expert_gate.append(gv)

    counts = [len(t) for t in expert_tok]
    NG, MG, T = _pick_shape(counts)
    NI = NG * MG

    key = (NG, MG, T)
    if key not in _compiled:
        _compiled[key] = _build_program(NG, MG, T)
    nc = _compiled[key]

    # expert -> (core, group): expert 2c+g goes to core c, group g
    in_maps = []
    for c in range(N_CORES):
        xT_b = np.zeros((NI, 128, DC, T), dtype=np.float32)
        wsT_b = np.empty((NG, CB, 128, DC, 128), dtype=np.float32)
        w2T_b = np.empty((NG, DC, 128, IC, 128), dtype=np.float32)
        for g in range(NG):
            e = NG * c + g
            wsT_b[g] = _tile_ws(ws[e])
            w2T_b[g] = _tile_w2(w2s[e])
            toks = expert_tok[e]
            for j in range(MG):
                seg = toks[j * T : (j + 1) * T]
                n = len(seg)
                if n == 0:
                    continue
                xT_b[g * MG + j, :, :, :n] = (
                    x[seg].reshape(n, DC, 128).transpose(2, 1, 0)
                )
        in_maps.append({"xT": xT_b, "wsT": wsT_b, "w2T": w2T_b})

    res = run_bass_kernel_spmd(
        nc, in_maps, core_ids=list(range(N_CORES)), trace=TRACE
    )
    LAST_EXEC_NS = res.exec_time_ns

    out = np.zeros((T_tok, D), dtype=np.float32)
    for e in range(E):
        c, g = divmod(e, NG)
        toks = expert_tok[e]
        gates = expert_gate[e]
        yT_c = res.results[c]["yT"]
        for j in range(MG):
            seg = toks[j * T : (j + 1) * T]
            n = len(seg)
            if n == 0:
                break
            y_item = yT_c[g * MG + j].transpose(2, 0, 1).reshape(T, D)[:n]
            out[seg] += gates[j * T : (j + 1) * T][:, None] * y_item
    return out


# revision 7
# speedup vs baseline: 1.2687x; 1.0036x over previous
"""DBRX MoE experts kernel for 8 Trainium2 NeuronCores.

Strategy (expert-parallel with host-side token dispatch):
  - Host computes the (cheap) router: softmax over 16 experts, top-4,
    renormalized gates.  Tokens are gathered per expert.
  - Each core gets NG=2 expert "groups" (16 experts / 8 cores).  Experts are
    sorted by token count: the 8 largest go in group 0, the 8 smallest in
    group 1, and each group's tokens are packed into MG tiles of T_g tokens
    (zero padded, T sized per group to the largest expert in it).  The
    expert's weights are loaded once per group and reused across its tiles.
  - Device (SPMD, one program on all 8 cores) runs the expert FFN:
    h = wsT.T @ x (both halves), act = silu(h1)*h2, y = w2T.T @ act.
    All matmuls in float32r (full PE speed at N>=256, ~2e-4 rel err).
  - Host applies gates and scatter-adds item outputs into the [T, D] output.
    Only the FFN (97% of the FLOPs) runs on device; the dense 16-expert
    reference computation is avoided entirely (4x FLOP saving via top-4).

Self-contained: hardcodes T=4096 tokens, D=1024, I=2048, E=16, top_k=4,
8 cores.
"""

import sys

if "/opt/trn_rl_repo" not in sys.path:
    sys.path.insert(0, "/opt/trn_rl_repo")

import numpy as np

import concourse.bacc as bacc
import concourse.mybir as mybir
import concourse.tile as tile
from concourse.bass_utils import run_bass_kernel_spmd

TOP_K = 4
N_CORES = 8
D = 1024
I = 2048
E = 16
DC = D // 128  # 8 contraction chunks for mm1 / output blocks for mm2
IC = I // 128  # 16 intermediate blocks
CB = 2 * I // 128  # 32 column blocks of ws

TRACE = False
LAST_EXEC_NS = None

_compiled = {}  # shapes tuple -> nc


def _build_program(shapes):
    """shapes: tuple of (MG, T) per group (one group = one expert)."""
    f32r = mybir.dt.float32r
    f32 = mybir.dt.float32
    NG = len(shapes)
    nc = bacc.Bacc("TRN2", target_bir_lowering=False, debug=False, num_devices=N_CORES)

    xTs, yTs = [], []
    for g, (MG, T) in enumerate(shapes):
        xTs.append(
            nc.dram_tensor(f"xT{g}", [MG, 128, DC, T], f32r, kind="ExternalInput")
        )
        yTs.append(
            nc.dram_tensor(f"yT{g}", [MG, DC, 128, T], f32, kind="ExternalOutput")
        )
    wsT = nc.dram_tensor("wsT", [NG, CB, 128, DC, 128], f32r, kind="ExternalInput")
    w2T = nc.dram_tensor("w2T", [NG, DC, 128, IC, 128], f32r, kind="ExternalInput")

    with tile.TileContext(nc) as tc:
        with (
            tc.tile_pool(name="xp", bufs=max(mg for mg, _ in shapes) + 1) as xp,
            tc.tile_pool(name="wp", bufs=4) as wp,
            tc.tile_pool(name="w2p", bufs=3) as w2p,
            tc.tile_pool(name="actp", bufs=max(mg for mg, _ in shapes) + 1) as actp,
            tc.tile_pool(name="sp", bufs=4) as sp,
            tc.tile_pool(name="pp", bufs=6, space="PSUM") as pp,
            tc.tile_pool(name="pp2", bufs=2, space="PSUM") as pp2,
        ):
            Tmax = max(t for _, t in shapes)
            for g, (MG, T) in enumerate(shapes):
                # first weight pair ahead of the (larger) x loads; weights on
                # the sync queue, x chunks on the scalar queue so their issue
                # streams run in parallel
                w1t0 = wp.tile([128, DC, 128], f32r, tag="ws", name=f"w1t0_{g}")
                nc.sync.dma_start(w1t0[:], wsT.ap()[g, 0])
                v1t0 = wp.tile([128, DC, 128], f32r, tag="ws", name=f"v1t0_{g}")
                nc.sync.dma_start(v1t0[:], wsT.ap()[g, IC])
                xts = []
                acts = []
                for j in range(MG):
                    # allocate at Tmax with a shared tag (slot reuse across
                    # groups), slice to this group's T
                    xt = xp.tile(
                        [128, DC, Tmax], f32r, tag="x", name=f"x_{g}_{j}"
                    )[:, :, :T]
                    # split per d-chunk: one big DMA bottlenecks on a single
                    # DMA queue (~90 GB/s); 8 chunk DMAs spread across queues
                    for dc in range(DC):
                        nc.scalar.dma_start(xt[:, dc], xTs[g].ap()[j, :, dc])
                    xts.append(xt)
                    act = actp.tile(
                        [128, IC, Tmax], f32r, tag="act", name=f"act_{g}_{j}"
                    )[:, :, :T]
                    acts.append(act)

                # mm1 + SwiGLU: weights outer, token tiles inner (weight reuse)
                for ip in range(IC):
                    if ip == 0:
                        w1t, v1t = w1t0, v1t0
                    else:
                        w1t = wp.tile([128, DC, 128], f32r, tag="ws")
                        nc.sync.dma_start(w1t[:], wsT.ap()[g, ip])
                        v1t = wp.tile([128, DC, 128], f32r, tag="ws")
                        nc.sync.dma_start(v1t[:], wsT.ap()[g, IC + ip])
                    for j in range(MG):
                        ps1 = pp.tile([128, T], f32, tag="h")
                        ps2 = pp.tile([128, T], f32, tag="h")
                        for dc in range(DC):
                            nc.tensor.matmul(
                                ps1[:], w1t[:, dc], xts[j][:, dc],
                                start=(dc == 0), stop=(dc == DC - 1),
                            )
                        for dc in range(DC):
                            nc.tensor.matmul(
                                ps2[:], v1t[:, dc], xts[j][:, dc],
                                start=(dc == 0), stop=(dc == DC - 1),
                            )
                        st = sp.tile([128, T], f32, tag="silu")
                        nc.scalar.activation(
                            st[:], ps1[:], mybir.ActivationFunctionType.Silu
                        )
                        nc.vector.tensor_mul(acts[j][:, ip], st[:], ps2[:])

                # mm2: w2 slabs outer, token tiles inner (weight reuse)
                for db in range(DC):
                    w2t = w2p.tile([128, IC, 128], f32r, tag="w2")
                    nc.sync.dma_start(w2t[:], w2T.ap()[g, db])
                    for j in range(MG):
                        ps3 = pp2.tile([128, T], f32, tag="y")
                        for ic in range(IC):
                            nc.tensor.matmul(
                                ps3[:], w2t[:, ic], acts[j][:, ic],
                                start=(ic == 0), stop=(ic == IC - 1),
                            )
                        ot = sp.tile([128, T], f32, tag="yout")
                        nc.any.tensor_copy(ot[:], ps3[:])
                        nc.sync.dma_start(yTs[g].ap()[j, db], ot[:])
    nc.compile()
    return nc


def _routing(x, rw):
    logits = x @ rw.T
    m = logits.max(-1, keepdims=True)
    p = np.exp(logits - m)
    p /= p.sum(-1, keepdims=True)
    topk_idx = np.argpartition(-p, TOP_K - 1, axis=-1)[:, :TOP_K]
    topk_val = np.take_along_axis(p, topk_idx, -1)
    topk_val = topk_val / topk_val.sum(-1, keepdims=True)
    return topk_idx, topk_val


def _group_shape(cmax):
    """Pick (MG, T) so MG*T >= cmax, T in [256, 512], minimizing MG*T."""
    best = None
    for MG in range(1, 17):
        T = -(-cmax // MG) if cmax else 256
        T = (T + 7) // 8 * 8
        if T > 512:
            continue
        T = max(T, 256)  # float32r matmul needs N>=256 for full PE speed
        if best is None or MG * T < best[0]:
            best = (MG * T, MG, T)
    assert best is not None
    return best[1], best[2]


def _tile_ws(ws_e):
    # [cb, p, dc, col] = ws_e[cb*128+col, dc*128+p]
    return np.ascontiguousarray(
        ws_e.reshape(CB, 128, DC, 128).transpose(0, 3, 2, 1)
    )


def _tile_w2(w2_e):
    # [db, p, ic, col] = w2_e[db*128+col, ic*128+p]
    return np.ascontiguousarray(
        w2_e.reshape(DC, 128, IC, 128).transpose(0, 3, 2, 1)
    )


def kernel(hidden_states, router_w, ws, w2s):
    global LAST_EXEC_NS
    x = np.ascontiguousarray(np.asarray(hidden_states, dtype=np.float32))
    rw = np.asarray(router_w, dtype=np.float32)
    ws = np.asarray(ws, dtype=np.float32)
    w2s = np.asarray(w2s, dtype=np.float32)
    T_tok = x.shape[0]

    topk_idx, topk_val = _routing(x, rw)

    expert_tok = []
    expert_gate = []
    for e in range(E):
        hit = topk_idx == e
        rows = np.nonzero(hit.any(-1))[0]
        gv = np.where(hit[rows], topk_val[rows], 0.0).sum(-1).astype(np.float32)
        expert_tok.append(rows)
        expert_gate.append(gv)

    counts = np.array([len(t) for t in expert_tok])
    NG = -(-E // N_CORES)  # 2
    # sort experts by count desc; group g holds ranks [g*8, g*8+8)
    order = np.argsort(-counts, kind="stable")
    groups = [order[g * N_CORES : (g + 1) * N_CORES] for g in range(NG)]
    # one MG for all groups (shared SBUF pool slots), per-group T
    MG0, _ = _group_shape(int(counts.max()))
    shapes = []
    for grp in groups:
        cmax = int(counts[grp].max()) if len(grp) else 0
        T = max(256, (-(-cmax // MG0) + 7) // 8 * 8)
        assert T <= 512
        shapes.append((MG0, T))
    shapes = tuple(shapes)

    if shapes not in _compiled:
        _compiled[shapes] = _build_program(shapes)
    nc = _compiled[shapes]

    in_maps = []
    for c in range(N_CORES):
        m = {}
        wsT_b = np.empty((NG, CB, 128, DC, 128), dtype=np.float32)
        w2T_b = np.empty((NG, DC, 128, IC, 128), dtype=np.float32)
        for g, (MG, T) in enumerate(shapes):
            e = int(groups[g][c])
            wsT_b[g] = _tile_ws(ws[e])
            w2T_b[g] = _tile_w2(w2s[e])
            xT_b = np.zeros((MG, 128, DC, T), dtype=np.float32)
            toks = expert_tok[e]
            for j in range(MG):
                seg = toks[j * T : (j + 1) * T]
                n = len(seg)
                if n == 0:
                    continue
                xT_b[j, :, :, :n] = x[seg].reshape(n, DC, 128).transpose(2, 1, 0)
            m[f"xT{g}"] = xT_b
        m["wsT"] = wsT_b
        m["w2T"] = w2T_b
        in_maps.append(m)

    res = run_bass_kernel_spmd(
        nc, in_maps, core_ids=list(range(N_CORES)), trace=TRACE
    )
    LAST_EXEC_NS = res.exec_time_ns

    out = np.zeros((T_tok, D), dtype=np.float32)
    for g, (MG, T) in enumerate(shapes):
        for c in range(N_CORES):
            e = int(groups[g][c])
            toks = expert_tok[e]
            gates = expert_gate[e]
            yT_c = res.results[c][f"yT{g}"]
            for j in range(MG):
                seg = toks[j * T : (j + 1) * T]
                n = len(seg)
                if n == 0:
                    break
                y_item = yT_c[j].transpose(2, 0, 1).reshape(T, D)[:n]
                out[seg] += gates[j * T : (j + 1) * T][:, None] * y_item
    return out


# revision 9
# speedup vs baseline: 1.2870x; 1.0144x over previous
"""DBRX MoE experts kernel for 8 Trainium2 NeuronCores.

Strategy (expert-parallel with host-side token dispatch):
  - Host computes the (cheap) router: softmax over 16 experts, top-4,
    renormalized gates.  Tokens are gathered per expert.
  - Each core gets NG=2 expert "groups" (16 experts / 8 cores).  Experts are
    sorted by token count: the 8 largest go in group 0, the 8 smallest in
    group 1, and each group's tokens are packed into MG tiles of T_g tokens
    (zero padded, T sized per group to the largest expert in it).  The
    expert's weights are loaded once per group and reused across its tiles.
  - Device (SPMD, one program on all 8 cores) runs the expert FFN:
    h = wsT.T @ x (both halves), act = silu(h1)*h2, y = w2T.T @ act.
    All matmuls in float32r (full PE speed at N>=256, ~2e-4 rel err).
  - Host applies gates and scatter-adds item outputs into the [T, D] output.
    Only the FFN (97% of the FLOPs) runs on device; the dense 16-expert
    reference computation is avoided entirely (4x FLOP saving via top-4).

Self-contained: hardcodes T=4096 tokens, D=1024, I=2048, E=16, top_k=4,
8 cores.
"""

import sys

if "/opt/trn_rl_repo" not in sys.path:
    sys.path.insert(0, "/opt/trn_rl_repo")

import numpy as np

import concourse.bacc as bacc
import concourse.mybir as mybir
import concourse.tile as tile
from concourse.bass_utils import run_bass_kernel_spmd

TOP_K = 4
N_CORES = 8
D = 1024
I = 2048
E = 16
DC = D // 128  # 8 contraction chunks for mm1 / output blocks for mm2
IC = I // 128  # 16 intermediate blocks
CB = 2 * I // 128  # 32 column blocks of ws

TRACE = False
LAST_EXEC_NS = None

_compiled = {}  # shapes tuple -> nc


def _build_program(shapes):
    """shapes: tuple of (MG, T) per group (one group = one expert)."""
    f32r = mybir.dt.float32r
    f32 = mybir.dt.float32
    NG = len(shapes)
    nc = bacc.Bacc("TRN2", target_bir_lowering=False, debug=False, num_devices=N_CORES)

    xTs, yTs = [], []
    for g, (MG, T) in enumerate(shapes):
        xTs.append(
            nc.dram_tensor(f"xT{g}", [MG, 128, DC, T], f32r, kind="ExternalInput")
        )
        yTs.append(
            nc.dram_tensor(f"yT{g}", [MG, DC, 128, T], f32, kind="ExternalOutput")
        )
    wsT = nc.dram_tensor("wsT", [NG, CB, 128, DC, 128], f32r, kind="ExternalInput")
    w2T = nc.dram_tensor("w2T", [NG, DC, 128, IC, 128], f32r, kind="ExternalInput")

    with tile.TileContext(nc) as tc:
        with (
            tc.tile_pool(name="xp", bufs=max(mg for mg, _ in shapes) + 1) as xp,
            tc.tile_pool(name="wp", bufs=6) as wp,
            tc.tile_pool(name="w2p", bufs=3) as w2p,
            tc.tile_pool(name="actp", bufs=max(mg for mg, _ in shapes)) as actp,
            tc.tile_pool(name="sp", bufs=4) as sp,
            tc.tile_pool(name="pp", bufs=6, space="PSUM") as pp,
            tc.tile_pool(name="pp2", bufs=2, space="PSUM") as pp2,
        ):
            Tmax = max(t for _, t in shapes)
            for g, (MG, T) in enumerate(shapes):
                # first weight pair ahead of the (larger) x loads; weights on
                # the sync queue, x chunks on the scalar queue so their issue
                # streams run in parallel
                w1t0 = wp.tile([128, DC, 128], f32r, tag="ws", name=f"w1t0_{g}")
                nc.sync.dma_start(w1t0[:], wsT.ap()[g, 0])
                v1t0 = wp.tile([128, DC, 128], f32r, tag="ws", name=f"v1t0_{g}")
                nc.sync.dma_start(v1t0[:], wsT.ap()[g, IC])
                xts = []
                acts = []
                for j in range(MG):
                    # allocate at Tmax with a shared tag (slot reuse across
                    # groups), slice to this group's T
                    xt = xp.tile(
                        [128, DC, Tmax], f32r, tag="x", name=f"x_{g}_{j}"
                    )[:, :, :T]
                    # split into 4 chunks: one big DMA bottlenecks on a single
                    # DMA queue (~90 GB/s); chunk DMAs spread across queues
                    # (with ~0.6us issue cost each, 4 is the sweet spot)
                    for dc in range(0, DC, 2):
                        nc.scalar.dma_start(
                            xt[:, dc : dc + 2], xTs[g].ap()[j, :, dc : dc + 2]
                        )
                    xts.append(xt)
                    act = actp.tile(
                        [128, IC, Tmax], f32r, tag="act", name=f"act_{g}_{j}"
                    )[:, :, :T]
                    acts.append(act)

                # mm1 + SwiGLU: weights outer, token tiles inner (weight reuse)
                for ip in range(IC):
                    if ip == 0:
                        w1t, v1t = w1t0, v1t0
                    else:
                        w1t = wp.tile([128, DC, 128], f32r, tag="ws")
                        nc.sync.dma_start(w1t[:], wsT.ap()[g, ip])
                        v1t = wp.tile([128, DC, 128], f32r, tag="ws")
                        nc.sync.dma_start(v1t[:], wsT.ap()[g, IC + ip])
                    for j in range(MG):
                        ps1 = pp.tile([128, T], f32, tag="h")
                        ps2 = pp.tile([128, T], f32, tag="h")
                        for dc in range(DC):
                            nc.tensor.matmul(
                                ps1[:], w1t[:, dc], xts[j][:, dc],
                                start=(dc == 0), stop=(dc == DC - 1),
                            )
                        for dc in range(DC):
                            nc.tensor.matmul(
                                ps2[:], v1t[:, dc], xts[j][:, dc],
                                start=(dc == 0), stop=(dc == DC - 1),
                            )
                        st = sp.tile([128, T], f32, tag="silu")
                        nc.scalar.activation(
                            st[:], ps1[:], mybir.ActivationFunctionType.Silu
                        )
                        nc.vector.tensor_mul(acts[j][:, ip], st[:], ps2[:])

                # mm2: w2 slabs outer, token tiles inner (weight reuse)
                for db in range(DC):
                    w2t = w2p.tile([128, IC, 128], f32r, tag="w2")
                    nc.sync.dma_start(w2t[:], w2T.ap()[g, db])
                    for j in range(MG):
                        ps3 = pp2.tile([128, T], f32, tag="y")
                        for ic in range(IC):
                            nc.tensor.matmul(
                                ps3[:], w2t[:, ic], acts[j][:, ic],
                                start=(ic == 0), stop=(ic == IC - 1),
                            )
                        ot = sp.tile([128, T], f32, tag="yout")
                        nc.any.tensor_copy(ot[:], ps3[:])
                        nc.sync.dma_start(yTs[g].ap()[j, db], ot[:])
    nc.compile()
    return nc


def _routing(x, rw):
    logits = x @ rw.T
    m = logits.max(-1, keepdims=True)
    p = np.exp(logits - m)
    p /= p.sum(-1, keepdims=True)
    topk_idx = np.argpartition(-p, TOP_K - 1, axis=-1)[:, :TOP_K]
    topk_val = np.take_along_axis(p, topk_idx, -1)
    topk_val = topk_val / topk_val.sum(-1, keepdims=True)
    return topk_idx, topk_val


def _group_shape(cmax):
    """Pick (MG, T) so MG*T >= cmax, T in [256, 512], minimizing MG*T."""
    best = None
    for MG in range(1, 17):
        T = -(-cmax // MG) if cmax else 256
        T = (T + 7) // 8 * 8
        if T > 512:
            continue
        T = max(T, 256)  # float32r matmul needs N>=256 for full PE speed
        if best is None or MG * T < best[0]:
            best = (MG * T, MG, T)
    assert best is not None
    return best[1], best[2]


def _tile_ws(ws_e):
    # [cb, p, dc, col] = ws_e[cb*128+col, dc*128+p]
    return np.ascontiguousarray(
        ws_e.reshape(CB, 128, DC, 128).transpose(0, 3, 2, 1)
    )


def _tile_w2(w2_e):
    # [db, p, ic, col] = w2_e[db*128+col, ic*128+p]
    return np.ascontiguousarray(
        w2_e.reshape(DC, 128, IC, 128).transpose(0, 3, 2, 1)
    )


def kernel(hidden_states, router_w, ws, w2s):
    global LAST_EXEC_NS
    x = np.ascontiguousarray(np.asarray(hidden_states, dtype=np.float32))
    rw = np.asarray(router_w, dtype=np.float32)
    ws = np.asarray(ws, dtype=np.float32)
    w2s = np.asarray(w2s, dtype=np.float32)
    T_tok = x.shape[0]

    topk_idx, topk_val = _routing(x, rw)

    expert_tok = []
    expert_gate = []
    for e in range(E):
        hit = topk_idx == e
        rows = np.nonzero(hit.any(-1))[0]
        gv = np.where(hit[rows], topk_val[rows], 0.0).sum(-1).astype(np.float32)
        expert_tok.append(rows)
        expert_gate.append(gv)

    counts = np.array([len(t) for t in expert_tok])
    NG = -(-E // N_CORES)  # 2
    # sort experts by count desc; group g holds ranks [g*8, g*8+8)
    order = np.argsort(-counts, kind="stable")
    groups = [order[g * N_CORES : (g + 1) * N_CORES] for g in range(NG)]
    # one MG for all groups (shared SBUF pool slots), per-group T
    MG0, _ = _group_shape(int(counts.max()))
    shapes = []
    for grp in groups:
        cmax = int(counts[grp].max()) if len(grp) else 0
        T = max(256, (-(-cmax // MG0) + 7) // 8 * 8)
        assert T <= 512
        shapes.append((MG0, T))
    shapes = tuple(shapes)

    if shapes not in _compiled:
        _compiled[shapes] = _build_program(shapes)
    nc = _compiled[shapes]

    in_maps = []
    for c in range(N_CORES):
        m = {}
        wsT_b = np.empty((NG, CB, 128, DC, 128), dtype=np.float32)
        w2T_b = np.empty((NG, DC, 128, IC, 128), dtype=np.float32)
        for g, (MG, T) in enumerate(shapes):
            e = int(groups[g][c])
            wsT_b[g] = _tile_ws(ws[e])
            w2T_b[g] = _tile_w2(w2s[e])
            xT_b = np.zeros((MG, 128, DC, T), dtype=np.float32)
            toks = expert_tok[e]
            for j in range(MG):
                seg = toks[j * T : (j + 1) * T]
                n = len(seg)
                if n == 0:
                    continue
                xT_b[j, :, :, :n] = x[seg].reshape(n, DC, 128).transpose(2, 1, 0)
            m[f"xT{g}"] = xT_b
        m["wsT"] = wsT_b
        m["w2T"] = w2T_b
        in_maps.append(m)

    res = run_bass_kernel_spmd(
        nc, in_maps, core_ids=list(range(N_CORES)), trace=TRACE
    )
    LAST_EXEC_NS = res.exec_time_ns

    out = np.zeros((T_tok, D), dtype=np.float32)
    for g, (MG, T) in enumerate(shapes):
        for c in range(N_CORES):
            e = int(groups[g][c])
            toks = expert_tok[e]
            gates = expert_gate[e]
            yT_c = res.results[c][f"yT{g}"]
            for j in range(MG):
                seg = toks[j * T : (j + 1) * T]
                n = len(seg)
                if n == 0:
                    break
                y_item = yT_c[j].transpose(2, 0, 1).reshape(T, D)[:n]
                out[seg] += gates[j * T : (j + 1) * T][:, None] * y_item
    return out


# revision 10
# speedup vs baseline: 1.3052x; 1.0142x over previous
"""DBRX MoE experts kernel for 8 Trainium2 NeuronCores.

Strategy (expert-parallel with host-side token dispatch):
  - Host computes the (cheap) router: softmax over 16 experts, top-4,
    renormalized gates.  Tokens are gathered per expert.
  - Each core gets NG=2 expert "groups" (16 experts / 8 cores).  Experts are
    sorted by token count: the 8 largest go in group 0, the 8 smallest in
    group 1, and each group's tokens are packed into MG tiles of T_g tokens
    (zero padded, T sized per group to the largest expert in it).  The
    expert's weights are loaded once per group and reused across its tiles.
  - Device (SPMD, one program on all 8 cores) runs the expert FFN:
    h = wsT.T @ x (both halves), act = silu(h1)*h2, y = w2T.T @ act.
    All matmuls in float32r (full PE speed at N>=256, ~2e-4 rel err).
  - Host applies gates and scatter-adds item outputs into the [T, D] output.
    Only the FFN (97% of the FLOPs) runs on device; the dense 16-expert
    reference computation is avoided entirely (4x FLOP saving via top-4).

Self-contained: hardcodes T=4096 tokens, D=1024, I=2048, E=16, top_k=4,
8 cores.
"""

import sys

if "/opt/trn_rl_repo" not in sys.path:
    sys.path.insert(0, "/opt/trn_rl_repo")

import numpy as np

import concourse.bacc as bacc
import concourse.mybir as mybir
import concourse.tile as tile
from concourse.bass_utils import run_bass_kernel_spmd

TOP_K = 4
N_CORES = 8
D = 1024
I = 2048
E = 16
DC = D // 128  # 8 contraction chunks for mm1 / output blocks for mm2
IC = I // 128  # 16 intermediate blocks
CB = 2 * I // 128  # 32 column blocks of ws

TRACE = False
LAST_EXEC_NS = None

_compiled = {}  # shapes tuple -> nc


def _build_program(shapes):
    """shapes: tuple of (MG, T) per group (one group = one expert)."""
    f32r = mybir.dt.float32r
    f32 = mybir.dt.float32
    NG = len(shapes)
    nc = bacc.Bacc("TRN2", target_bir_lowering=False, debug=False, num_devices=N_CORES)

    xTs, yTs = [], []
    for g, (MG, T) in enumerate(shapes):
        xTs.append(
            nc.dram_tensor(f"xT{g}", [MG, 128, DC, T], f32r, kind="ExternalInput")
        )
        yTs.append(
            nc.dram_tensor(f"yT{g}", [MG, DC, 128, T], f32, kind="ExternalOutput")
        )
    wsT = nc.dram_tensor("wsT", [NG, CB, 128, DC, 128], f32r, kind="ExternalInput")
    w2T = nc.dram_tensor("w2T", [NG, DC, 128, IC, 128], f32r, kind="ExternalInput")

    with tile.TileContext(nc) as tc:
        with (
            tc.tile_pool(name="xp", bufs=max(mg for mg, _ in shapes) + 1) as xp,
            tc.tile_pool(name="wp", bufs=6) as wp,
            tc.tile_pool(name="w2p", bufs=3) as w2p,
            tc.tile_pool(name="actp", bufs=max(mg for mg, _ in shapes)) as actp,
            tc.tile_pool(name="sp", bufs=4) as sp,
            tc.tile_pool(name="pp", bufs=6, space="PSUM") as pp,
            tc.tile_pool(name="pp2", bufs=2, space="PSUM") as pp2,
        ):
            Tmax = max(t for _, t in shapes)
            for g, (MG, T) in enumerate(shapes):
                # first weight pair ahead of the (larger) x loads; weights on
                # the sync queue, x chunks on the scalar queue so their issue
                # streams run in parallel
                w1t0 = wp.tile([128, DC, 128], f32r, tag="ws", name=f"w1t0_{g}")
                nc.sync.dma_start(w1t0[:], wsT.ap()[g, 0])
                v1t0 = wp.tile([128, DC, 128], f32r, tag="ws", name=f"v1t0_{g}")
                nc.sync.dma_start(v1t0[:], wsT.ap()[g, IC])
                xts = []
                acts = []
                for j in range(MG):
                    # allocate at Tmax with a shared tag (slot reuse across
                    # groups), slice to this group's T
                    xt = xp.tile(
                        [128, DC, Tmax], f32r, tag="x", name=f"x_{g}_{j}"
                    )[:, :, :T]
                    # split into 4 chunks: one big DMA bottlenecks on a single
                    # DMA queue (~90 GB/s); chunk DMAs spread across queues
                    # (with ~0.6us issue cost each, 4 is the sweet spot).
                    # Alternate the two HWDGE issue queues (sync/scalar) so
                    # the startup-critical transfers issue in parallel.
                    for k, dc in enumerate(range(0, DC, 2)):
                        eng = nc.scalar if (j + k) % 2 else nc.sync
                        eng.dma_start(
                            xt[:, dc : dc + 2], xTs[g].ap()[j, :, dc : dc + 2]
                        )
                    xts.append(xt)
                    act = actp.tile(
                        [128, IC, Tmax], f32r, tag="act", name=f"act_{g}_{j}"
                    )[:, :, :T]
                    acts.append(act)

                # mm1 + SwiGLU: weights outer, token tiles inner (weight reuse)
                for ip in range(IC):
                    if ip == 0:
                        w1t, v1t = w1t0, v1t0
                    else:
                        w1t = wp.tile([128, DC, 128], f32r, tag="ws")
                        nc.sync.dma_start(w1t[:], wsT.ap()[g, ip])
                        v1t = wp.tile([128, DC, 128], f32r, tag="ws")
                        nc.sync.dma_start(v1t[:], wsT.ap()[g, IC + ip])
                    for j in range(MG):
                        ps1 = pp.tile([128, T], f32, tag="h")
                        ps2 = pp.tile([128, T], f32, tag="h")
                        for dc in range(DC):
                            nc.tensor.matmul(
                                ps1[:], w1t[:, dc], xts[j][:, dc],
                                start=(dc == 0), stop=(dc == DC - 1),
                            )
                        for dc in range(DC):
                            nc.tensor.matmul(
                                ps2[:], v1t[:, dc], xts[j][:, dc],
                                start=(dc == 0), stop=(dc == DC - 1),
                            )
                        st = sp.tile([128, T], f32, tag="silu")
                        nc.scalar.activation(
                            st[:], ps1[:], mybir.ActivationFunctionType.Silu
                        )
                        nc.vector.tensor_mul(acts[j][:, ip], st[:], ps2[:])

                # mm2: w2 slabs outer, token tiles inner (weight reuse)
                for db in range(DC):
                    w2t = w2p.tile([128, IC, 128], f32r, tag="w2")
                    nc.sync.dma_start(w2t[:], w2T.ap()[g, db])
                    for j in range(MG):
                        ps3 = pp2.tile([128, T], f32, tag="y")
                        for ic in range(IC):
                            nc.tensor.matmul(
                                ps3[:], w2t[:, ic], acts[j][:, ic],
                                start=(ic == 0), stop=(ic == IC - 1),
                            )
                        ot = sp.tile([128, T], f32, tag="yout")
                        nc.any.tensor_copy(ot[:], ps3[:])
                        nc.sync.dma_start(yTs[g].ap()[j, db], ot[:])
    nc.compile()
    return nc


def _routing(x, rw):
    logits = x @ rw.T
    m = logits.max(-1, keepdims=True)
    p = np.exp(logits - m)
    p /= p.sum(-1, keepdims=True)
    topk_idx = np.argpartition(-p, TOP_K - 1, axis=-1)[:, :TOP_K]
    topk_val = np.take_along_axis(p, topk_idx, -1)
    topk_val = topk_val / topk_val.sum(-1, keepdims=True)
    return topk_idx, topk_val


def _group_shape(cmax):
    """Pick (MG, T) so MG*T >= cmax, T in [256, 512], minimizing MG*T."""
    best = None
    for MG in range(1, 17):
        T = -(-cmax // MG) if cmax else 256
        T = (T + 7) // 8 * 8
        if T > 512:
            continue
        T = max(T, 256)  # float32r matmul needs N>=256 for full PE speed
        if best is None or MG * T < best[0]:
            best = (MG * T, MG, T)
    assert best is not None
    return best[1], best[2]


def _tile_ws(ws_e):
    # [cb, p, dc, col] = ws_e[cb*128+col, dc*128+p]
    return np.ascontiguousarray(
        ws_e.reshape(CB, 128, DC, 128).transpose(0, 3, 2, 1)
    )


def _tile_w2(w2_e):
    # [db, p, ic, col] = w2_e[db*128+col, ic*128+p]
    return np.ascontiguousarray(
        w2_e.reshape(DC, 128, IC, 128).transpose(0, 3, 2, 1)
    )


def kernel(hidden_states, router_w, ws, w2s):
    global LAST_EXEC_NS
    x = np.ascontiguousarray(np.asarray(hidden_states, dtype=np.float32))
    rw = np.asarray(router_w, dtype=np.float32)
    ws = np.asarray(ws, dtype=np.float32)
    w2s = np.asarray(w2s, dtype=np.float32)
    T_tok = x.shape[0]

    topk_idx, topk_val = _routing(x, rw)

    expert_tok = []
    expert_gate = []
    for e in range(E):
        hit = topk_idx == e
        rows = np.nonzero(hit.any(-1))[0]
        gv = np.where(hit[rows], topk_val[rows], 0.0).sum(-1).astype(np.float32)
        expert_tok.append(rows)
        expert_gate.append(gv)

    counts = np.array([len(t) for t in expert_tok])
    NG = -(-E // N_CORES)  # 2
    # sort experts by count desc; group g holds ranks [g*8, g*8+8)
    order = np.argsort(-counts, kind="stable")
    groups = [order[g * N_CORES : (g + 1) * N_CORES] for g in range(NG)]
    # one MG for all groups (shared SBUF pool slots), per-group T
    MG0, _ = _group_shape(int(counts.max()))
    shapes = []
    for grp in groups:
        cmax = int(counts[grp].max()) if len(grp) else 0
        T = max(256, (-(-cmax // MG0) + 7) // 8 * 8)
        assert T <= 512
        shapes.append((MG0, T))
    shapes = tuple(shapes)

    if shapes not in _compiled:
        _compiled[shapes] = _build_program(shapes)
    nc = _compiled[shapes]

    in_maps = []
    for c in range(N_CORES):
        m = {}
        wsT_b = np.empty((NG, CB, 128, DC, 128), dtype=np.float32)
        w2T_b = np.empty((NG, DC, 128, IC, 128), dtype=np.float32)
        for g, (MG, T) in enumerate(shapes):
            e = int(groups[g][c])
            wsT_b[g] = _tile_ws(ws[e])
            w2T_b[g] = _tile_w2(w2s[e])
            xT_b = np.zeros((MG, 128, DC, T), dtype=np.float32)
            toks = expert_tok[e]
            for j in range(MG):
                seg = toks[j * T : (j + 1) * T]
                n = len(seg)
                if n == 0:
                    continue
                xT_b[j, :, :, :n] = x[seg].reshape(n, DC, 128).transpose(2, 1, 0)
            m[f"xT{g}"] = xT_b
        m["wsT"] = wsT_b
        m["w2T"] = w2T_b
        in_maps.append(m)

    res = run_bass_kernel_spmd(
        nc, in_maps, core_ids=list(range(N_CORES)), trace=TRACE
    )
    LAST_EXEC_NS = res.exec_time_ns

    out = np.zeros((T_tok, D), dtype=np.float32)
    for g, (MG, T) in enumerate(shapes):
        for c in range(N_CORES):
            e = int(groups[g][c])
            toks = expert_tok[e]
            gates = expert_gate[e]
            yT_c = res.results[c][f"yT{g}"]
            for j in range(MG):
                seg = toks[j * T : (j + 1) * T]
                n = len(seg)
                if n == 0:
                    break
                y_item = yT_c[j].transpose(2, 0, 1).reshape(T, D)[:n]
                out[seg] += gates[j * T : (j + 1) * T][:, None] * y_item
    return out


# revision 12
# speedup vs baseline: 1.3389x; 1.0258x over previous
"""DBRX MoE experts kernel for 8 Trainium2 NeuronCores.

Strategy (expert-parallel with host-side token dispatch):
  - Host computes the (cheap) router: softmax over 16 experts, top-4,
    renormalized gates.  Tokens are gathered per expert.
  - Each core gets NG=2 expert "groups" (16 experts / 8 cores).  Experts are
    sorted by token count: the 8 largest go in group 0, the 8 smallest in
    group 1, and each group's tokens are packed into MG tiles of T_g tokens
    (zero padded, T sized per group to the largest expert in it).  The
    expert's weights are loaded once per group and reused across its tiles.
  - Device (SPMD, one program on all 8 cores) runs the expert FFN:
    h = wsT.T @ x (both halves), act = silu(h1)*h2, y = w2T.T @ act.
    All matmuls in float32r (full PE speed at N>=256, ~2e-4 rel err).
  - Host applies gates and scatter-adds item outputs into the [T, D] output.
    Only the FFN (97% of the FLOPs) runs on device; the dense 16-expert
    reference computation is avoided entirely (4x FLOP saving via top-4).

Self-contained: hardcodes T=4096 tokens, D=1024, I=2048, E=16, top_k=4,
8 cores.
"""

import sys

if "/opt/trn_rl_repo" not in sys.path:
    sys.path.insert(0, "/opt/trn_rl_repo")

import numpy as np

import concourse.bacc as bacc
import concourse.mybir as mybir
import concourse.tile as tile
from concourse.bass_utils import run_bass_kernel_spmd

TOP_K = 4
N_CORES = 8
D = 1024
I = 2048
E = 16
DC = D // 128  # 8 contraction chunks for mm1 / output blocks for mm2
IC = I // 128  # 16 intermediate blocks
CB = 2 * I // 128  # 32 column blocks of ws

TRACE = False
LAST_EXEC_NS = None

_compiled = {}  # shapes tuple -> nc


def _build_program(shapes):
    """shapes: tuple of (MG, T) per group (one group = one expert)."""
    f32r = mybir.dt.float32r
    f32 = mybir.dt.float32
    NG = len(shapes)
    nc = bacc.Bacc("TRN2", target_bir_lowering=False, debug=False, num_devices=N_CORES)

    xTs, yTs = [], []
    for g, (MG, T) in enumerate(shapes):
        xTs.append(
            nc.dram_tensor(f"xT{g}", [MG, 128, DC, T], f32r, kind="ExternalInput")
        )
        yTs.append(
            nc.dram_tensor(f"yT{g}", [MG, DC, 128, T], f32, kind="ExternalOutput")
        )
    wsT = nc.dram_tensor("wsT", [NG, CB, 128, DC, 128], f32r, kind="ExternalInput")
    w2T = nc.dram_tensor("w2T", [NG, DC, 128, IC, 128], f32r, kind="ExternalInput")

    with tile.TileContext(nc) as tc:
        with (
            tc.tile_pool(name="xp", bufs=max(mg for mg, _ in shapes)) as xp,
            tc.tile_pool(name="wp", bufs=4) as wp,
            tc.tile_pool(name="w2p", bufs=3) as w2p,
            tc.tile_pool(name="actp", bufs=max(mg for mg, _ in shapes)) as actp,
            tc.tile_pool(name="sp", bufs=3) as sp,
            tc.tile_pool(name="pp", bufs=6, space="PSUM") as pp,
            tc.tile_pool(name="pp2", bufs=2, space="PSUM") as pp2,
        ):
            Tmax = max(t for _, t in shapes)
            for g, (MG, T) in enumerate(shapes):
                # first weight pair ahead of the (larger) x loads; weights on
                # the sync queue, x chunks on the scalar queue so their issue
                # streams run in parallel
                w1t0 = wp.tile([128, DC, 128], f32r, tag="ws", name=f"w1t0_{g}")
                nc.sync.dma_start(w1t0[:], wsT.ap()[g, 0])
                v1t0 = wp.tile([128, DC, 128], f32r, tag="ws", name=f"v1t0_{g}")
                nc.sync.dma_start(v1t0[:], wsT.ap()[g, IC])
                xts = []
                acts = []
                for j in range(MG):
                    # allocate at Tmax with a shared tag (slot reuse across
                    # groups), slice to this group's T
                    xt = xp.tile(
                        [128, DC, Tmax], f32r, tag="x", name=f"x_{g}_{j}"
                    )[:, :, :T]
                    # split into 4 chunks: one big DMA bottlenecks on a single
                    # DMA queue (~90 GB/s); chunk DMAs spread across queues
                    # (with ~0.6us issue cost each, 4 is the sweet spot).
                    # Alternate the two HWDGE issue queues (sync/scalar) so
                    # the startup-critical transfers issue in parallel.
                    for k, dc in enumerate(range(0, DC, 2)):
                        eng = nc.scalar if (j + k) % 2 else nc.sync
                        eng.dma_start(
                            xt[:, dc : dc + 2], xTs[g].ap()[j, :, dc : dc + 2]
                        )
                    xts.append(xt)
                    act = actp.tile(
                        [128, IC, Tmax], f32r, tag="act", name=f"act_{g}_{j}"
                    )[:, :, :T]
                    acts.append(act)

                # mm1 + SwiGLU: weights outer, token tiles inner (weight reuse)
                for ip in range(IC):
                    if ip == 0:
                        w1t, v1t = w1t0, v1t0
                    else:
                        w1t = wp.tile([128, DC, 128], f32r, tag="ws")
                        nc.sync.dma_start(w1t[:], wsT.ap()[g, ip])
                        v1t = wp.tile([128, DC, 128], f32r, tag="ws")
                        nc.sync.dma_start(v1t[:], wsT.ap()[g, IC + ip])
                    for j in range(MG):
                        ps1 = pp.tile([128, T], f32, tag="h")
                        ps2 = pp.tile([128, T], f32, tag="h")
                        for dc in range(DC):
                            nc.tensor.matmul(
                                ps1[:], w1t[:, dc], xts[j][:, dc],
                                start=(dc == 0), stop=(dc == DC - 1),
                            )
                        for dc in range(DC):
                            nc.tensor.matmul(
                                ps2[:], v1t[:, dc], xts[j][:, dc],
                                start=(dc == 0), stop=(dc == DC - 1),
                            )
                        st = sp.tile([128, T], f32, tag="silu")
                        nc.scalar.activation(
                            st[:], ps1[:], mybir.ActivationFunctionType.Silu
                        )
                        nc.vector.tensor_mul(acts[j][:, ip], st[:], ps2[:])

                # mm2: w2 slabs outer, token tiles inner (weight reuse)
                for db in range(DC):
                    w2t = w2p.tile([128, IC, 128], f32r, tag="w2")
                    nc.sync.dma_start(w2t[:], w2T.ap()[g, db])
                    for j in range(MG):
                        ps3 = pp2.tile([128, T], f32, tag="y")
                        for ic in range(IC):
                            nc.tensor.matmul(
                                ps3[:], w2t[:, ic], acts[j][:, ic],
                                start=(ic == 0), stop=(ic == IC - 1),
                            )
                        ot = sp.tile([128, T], f32, tag="yout")
                        nc.any.tensor_copy(ot[:], ps3[:])
                        nc.sync.dma_start(yTs[g].ap()[j, db], ot[:])
    nc.compile()
    return nc


def _routing(x, rw):
    logits = x @ rw.T
    m = logits.max(-1, keepdims=True)
    p = np.exp(logits - m)
    p /= p.sum(-1, keepdims=True)
    topk_idx = np.argpartition(-p, TOP_K - 1, axis=-1)[:, :TOP_K]
    topk_val = np.take_along_axis(p, topk_idx, -1)
    topk_val = topk_val / topk_val.sum(-1, keepdims=True)
    return topk_idx, topk_val


def _group_shape(cmax):
    """Pick (MG, T) so MG*T >= cmax, T in [256, 512], minimizing MG*T."""
    best = None
    for MG in range(1, 17):
        T = -(-cmax // MG) if cmax else 256
        T = (T + 7) // 8 * 8
        if T > 512:
            continue
        T = max(T, 256)  # float32r matmul needs N>=256 for full PE speed
        if best is None or MG * T < best[0]:
            best = (MG * T, MG, T)
    assert best is not None
    return best[1], best[2]


def _tile_ws(ws_e):
    # [cb, p, dc, col] = ws_e[cb*128+col, dc*128+p]
    return np.ascontiguousarray(
        ws_e.reshape(CB, 128, DC, 128).transpose(0, 3, 2, 1)
    )


def _tile_w2(w2_e):
    # [db, p, ic, col] = w2_e[db*128+col, ic*128+p]
    return np.ascontiguousarray(
        w2_e.reshape(DC, 128, IC, 128).transpose(0, 3, 2, 1)
    )


def kernel(hidden_states, router_w, ws, w2s):
    global LAST_EXEC_NS
    x = np.ascontiguousarray(np.asarray(hidden_states, dtype=np.float32))
    rw = np.asarray(router_w, dtype=np.float32)
    ws = np.asarray(ws, dtype=np.float32)
    w2s = np.asarray(w2s, dtype=np.float32)
    T_tok = x.shape[0]

    topk_idx, topk_val = _routing(x, rw)

    expert_tok = []
    expert_gate = []
    for e in range(E):
        hit = topk_idx == e
        rows = np.nonzero(hit.any(-1))[0]
        gv = np.where(hit[rows], topk_val[rows], 0.0).sum(-1).astype(np.float32)
        expert_tok.append(rows)
        expert_gate.append(gv)

    counts = np.array([len(t) for t in expert_tok])
    NG = -(-E // N_CORES)  # 2
    # sort experts by count desc; group g holds ranks [g*8, g*8+8)
    order = np.argsort(-counts, kind="stable")
    groups = [order[g * N_CORES : (g + 1) * N_CORES] for g in range(NG)]
    shapes = tuple(
        _group_shape(int(counts[grp].max()) if len(grp) else 0) for grp in groups
    )

    if shapes not in _compiled:
        _compiled[shapes] = _build_program(shapes)
    nc = _compiled[shapes]

    in_maps = []
    for c in range(N_CORES):
        m = {}
        wsT_b = np.empty((NG, CB, 128, DC, 128), dtype=np.float32)
        w2T_b = np.empty((NG, DC, 128, IC, 128), dtype=np.float32)
        for g, (MG, T) in enumerate(shapes):
            e = int(groups[g][c])
            wsT_b[g] = _tile_ws(ws[e])
            w2T_b[g] = _tile_w2(w2s[e])
            xT_b = np.zeros((MG, 128, DC, T), dtype=np.float32)
            toks = expert_tok[e]
            for j in range(MG):
                seg = toks[j * T : (j + 1) * T]
                n = len(seg)
                if n == 0:
                    continue
                xT_b[j, :, :, :n] = x[seg].reshape(n, DC, 128).transpose(2, 1, 0)
            m[f"xT{g}"] = xT_b
        m["wsT"] = wsT_b
        m["w2T"] = w2T_b
        in_maps.append(m)

    res = run_bass_kernel_spmd(
        nc, in_maps, core_ids=list(range(N_CORES)), trace=TRACE
    )
    LAST_EXEC_NS = res.exec_time_ns

    out = np.zeros((T_tok, D), dtype=np.float32)
    for g, (MG, T) in enumerate(shapes):
        for c in range(N_CORES):
            e = int(groups[g][c])
            toks = expert_tok[e]
            gates = expert_gate[e]
            yT_c = res.results[c][f"yT{g}"]
            for j in range(MG):
                seg = toks[j * T : (j + 1) * T]
                n = len(seg)
                if n == 0:
                    break
                y_item = yT_c[j].transpose(2, 0, 1).reshape(T, D)[:n]
                out[seg] += gates[j * T : (j + 1) * T][:, None] * y_item
    return out
